# revision 15
# baseline (speedup 1.0000x reference)
# MoE-routing kernel for Trainium2: out[b] = x[b] @ weight[y[b]] + bias[y[b]]
# x: [1024, 64, 1152] f32, y: [1024] int, weight: [1000, 1152, 128] f32,
# bias: [1000, 128] f32 -> out: [1024, 64, 128] f32.
#
# Strategy: sort samples by class so samples sharing a class share one weight
# load (1024 -> ~660 weight loads), then shard class-groups across 8 cores
# with an identical group-size multiset per core (SPMD requires one program).
# Per group (class c, m<=4 samples) the device computes a weight-stationary
# matmul: for each of 9 k-tiles, lhsT = W_c k-tile [128k, 128o] (stationary),
# rhs = x k-tile [128k, 64m] (moving), accumulating [128o, 64m] in PSUM f32.
# Weights travel in fp8-e3m4 (per-class scale), x in bf16 or fp8-e3m4
# (per-row scale), output in bf16; scales/bias are applied on host.
# Memory-bound: the kernel sits at the per-core HBM DMA roofline.

import numpy as np

B, N, HIDDEN = 1024, 64, 1152
NUM_CLASSES = 1000
OUT_DIM = 128
KT = HIDDEN // 128  # 9 k-tiles
NCORES = 8
S = B // NCORES  # 128 samples per core
MAXM = 4         # max samples per group (PSUM free dim 64*m <= 256)

X_FP8 = True     # x dtype: False -> bf16, True -> fp8 e3m4 (per-row scale)
FP8_MAX = 15.5 * 0.96

_cache = {}


def _plan(y):
    """Group samples by class, canonicalize group sizes so every core gets an
    identical multiset, deal groups to cores. Returns (order, cores, sizes)
    where order sorts samples by class, cores[c] is a list of
    (cls, start, size) groups in canonical order (sizes descending), and
    sizes is the per-core size sequence (identical for all cores)."""
    order = np.argsort(y, kind="stable")
    ys = y[order]
    bysize = {s: [] for s in (4, 3, 2, 1)}
    i = 0
    while i < B:
        j = i
        while j < B and ys[j] == ys[i]:
            j += 1
        c, n, k = int(ys[i]), j - i, i
        while n > 0:
            t = min(n, MAXM)
            if n - t == 1 and t > 1:
                t -= 1  # avoid trailing singleton: prefer (3,2) over (4,1)
            bysize[t].append((c, k, t))
            k += t
            n -= t
        i = j
    # make each size count divisible by NCORES by splitting groups
    for s in (4, 3, 2):
        for _ in range(len(bysize[s]) % NCORES):
            c, st, _sz = bysize[s].pop()
            if s == 4:
                bysize[2] += [(c, st, 2), (c, st + 2, 2)]
            elif s == 3:
                bysize[2].append((c, st, 2))
                bysize[1].append((c, st + 2, 1))
            else:
                bysize[1] += [(c, st, 1), (c, st + 1, 1)]
    assert len(bysize[1]) % NCORES == 0
    percore = {s: len(bysize[s]) // NCORES for s in (4, 3, 2, 1)}
    # interleave sizes uniformly so every DMA chunk sees a similar
    # weight-bytes/sample mix (avoids a weight-DMA wall late in the run)
    slots = []
    for s, ns in percore.items():
        slots += [((j + 0.5) / ns, s) for j in range(ns)]
    slots.sort(key=lambda t: (t[0], t[1]))
    sizeseq = [s for _pos, s in slots]
    cores = [[] for _ in range(NCORES)]
    idx = {s: 0 for s in (4, 3, 2, 1)}
    for s in sizeseq:
        for c in range(NCORES):
            cores[c].append(bysize[s][idx[s] * NCORES + c])
        idx[s] += 1
    sizes = [g[2] for g in cores[0]]
    assert sum(sizes) == S
    return order, cores, sizes


def _chunks(sizes):
    """Split the canonical group sequence into DMA chunks. Returns a list of
    (g0, ng, s0, ns): group range and sample range per chunk."""
    targets = [2, 4, 8, 16] + [24] * 100
    out = []
    g0 = s0 = 0
    ti = 0
    while g0 < len(sizes):
        tgt = targets[min(ti, len(targets) - 1)]
        g, ns = g0, 0
        while g < len(sizes) and ns < tgt:
            ns += sizes[g]
            g += 1
        out.append((g0, g - g0, s0, ns))
        g0, s0, ti = g, s0 + ns, ti + 1
    # taper the final chunks so the pipeline drain is short
    for lim in (12, 6, 2):
        if out and out[-1][3] > lim + 2:
            g0, ng, s0, ns = out.pop()
            gs, acc = g0, 0
            while acc < ns - lim and gs < g0 + ng - 1:
                acc += sizes[gs]
                gs += 1
            out.append((g0, gs - g0, s0, acc))
            out.append((gs, g0 + ng - gs, s0 + acc, ns - acc))
    return out


def _build_nc(sizes, chunks, x_fp8):
    import concourse.bass as bass
    import concourse.mybir as mybir
    from concourse.tile import TileContext

    nc = bass.Bass()
    f32 = mybir.dt.float32
    bf16 = mybir.dt.bfloat16
    fp8 = mybir.dt.float8e3
    xdt = fp8 if x_fp8 else bf16
    U = len(sizes)
    # x is laid out per-chunk ([chunk][k][s][n]) so every x DMA is one
    # contiguous run per partition
    Xd = nc.declare_dram_parameter("xin", [128, KT * S * N], xdt, isOutput=False)
    Wd = nc.declare_dram_parameter("win", [128, U * KT * OUT_DIM], fp8, isOutput=False)
    Od = nc.declare_dram_parameter("o", [128, S * N], bf16, isOutput=True)

    with TileContext(nc) as tc:
        with (
            tc.tile_pool(name="xp", bufs=5) as xp,
            tc.tile_pool(name="wp", bufs=5) as wp,
            tc.tile_pool(name="op", bufs=4) as op,
            tc.tile_pool(name="pp", bufs=8, space="PSUM") as pp,
        ):
            for ci, (g0, ng, s0, ns) in enumerate(chunks):
                xt = xp.tile([128, KT * ns * N], xdt, tag="xt")
                nc.scalar.dma_start(
                    out=xt, in_=Xd[:, s0 * KT * N : (s0 + ns) * KT * N]
                )
                wt = wp.tile([128, ng * KT * OUT_DIM], fp8, tag="wt")
                nc.sync.dma_start(
                    out=wt,
                    in_=Wd[:, g0 * KT * OUT_DIM : (g0 + ng) * KT * OUT_DIM],
                )
                ot = op.tile([128, ns * N], bf16, tag="ot")
                spos = 0
                for gl in range(ng):
                    m = sizes[g0 + gl]
                    ps = pp.tile([128, MAXM * N], f32)
                    for k in range(KT):
                        nc.tensor.matmul(
                            ps[:, : m * N],
                            wt[:, (gl * KT + k) * OUT_DIM : (gl * KT + k + 1) * OUT_DIM],
                            xt[:, (k * ns + spos) * N : (k * ns + spos + m) * N],
                            start=(k == 0),
                            stop=(k == KT - 1),
                        )
                    nc.vector.tensor_copy(
                        ot[:, spos * N : (spos + m) * N], ps[:, : m * N]
                    )
                    spos += m
                # HBM-write receipts would bubble the input HWDGE rings, so
                # outputs ride SWDGE; the last (tiny) one goes HWDGE to
                # avoid the Q7 drain latency in the kernel tail.
                if ci == len(chunks) - 1:
                    nc.scalar.dma_start(out=Od[:, s0 * N : (s0 + ns) * N], in_=ot)
                elif ci % 2 == 0:
                    nc.gpsimd.dma_start(out=Od[:, s0 * N : (s0 + ns) * N], in_=ot)
                else:
                    nc.sync.dma_start(out=Od[:, s0 * N : (s0 + ns) * N], in_=ot)

    _split_excess_waits(nc)
    nc.finalize()
    _split_excess_waits(nc)
    return nc


def _split_excess_waits(nc, max_waits=1):
    # walrus codegen rejects instructions with >max sync waits; Tile's tail
    # drain can carry several. Hoist the excess onto preceding no-ops.
    import concourse.mybir as mybir

    for f in nc.m.functions:
        for b in f.blocks:
            i = 0
            while i < len(b.instructions):
                inst = b.instructions[i]
                si = inst.sync_info
                if si is not None and len(si.on_wait) > max_waits:
                    excess = list(si.on_wait[:-max_waits])
                    si.on_wait = list(si.on_wait[-max_waits:])
                    for w in excess:
                        nop = mybir.InstNoOp(
                            name=nc.get_next_instruction_name(),
                            engine=inst.engine,
                            sync_info=mybir.SyncInfo(on_wait=[w], on_update=[]),
                            bass_nofuse=True,
                        )
                        nc.register_instruction(nop)
                        b.instructions.insert(i, nop)
                        i += 1
                i += 1


def kernel(x, y, weight, bias):
    import ml_dtypes
    from concourse.bass_utils import run_bass_kernel_spmd

    e3 = ml_dtypes.float8_e3m4
    bf16 = ml_dtypes.bfloat16
    x = np.ascontiguousarray(x, dtype=np.float32)
    weight = np.ascontiguousarray(weight, dtype=np.float32)
    yi = np.asarray(y).astype(np.int64)

    order, cores, sizes = _plan(yi)
    chunks = _chunks(sizes)
    key = (tuple(sizes), X_FP8)
    if _cache.get("key") != key:
        _cache["nc"] = _build_nc(sizes, chunks, X_FP8)
        _cache["key"] = key
    nc = _cache["nc"]
    U = len(sizes)

    # --- quantize x ---
    if X_FP8:
        sx = np.abs(x).max(axis=2) / FP8_MAX  # [B, N]
        np.maximum(sx, 1e-30, out=sx)
        xq = (x / sx[:, :, None]).astype(e3)
    else:
        sx = np.ones((B, N), dtype=np.float32)
        xq = x.astype(bf16)

    # --- quantize weights for used classes (per-class scale) ---
    used = sorted({g[0] for core in cores for g in core})
    cls_slot = {c: i for i, c in enumerate(used)}
    wu = weight[used]  # [Ucls, H, O]
    swu = np.abs(wu).max(axis=(1, 2)) / FP8_MAX  # [Ucls]
    np.maximum(swu, 1e-30, out=swu)
    wq = (wu / swu[:, None, None]).astype(e3)
    # [Ucls, KT, 128, O] -> [128, Ucls, KT, O]
    wq = np.ascontiguousarray(
        wq.reshape(len(used), KT, 128, OUT_DIM).transpose(2, 0, 1, 3)
    )

    # --- per-core input layouts ---
    in_maps = []
    core_meta = []
    for core in cores:
        ids = np.concatenate(
            [order[st : st + m] for (_c, st, m) in core]
        )  # original sample ids, canonical order
        slots = np.array([cls_slot[c] for (c, _st, _m) in core])
        # x: per chunk [ns, N, KT, 128] -> [128, KT, ns, N], chunks concatenated
        xh = np.empty((128, KT * S * N), dtype=xq.dtype)
        for (_g0, _ng, s0, ns) in chunks:
            xh[:, s0 * KT * N : (s0 + ns) * KT * N] = (
                xq[ids[s0 : s0 + ns]]
                .reshape(ns, N, KT, 128)
                .transpose(3, 2, 0, 1)
                .reshape(128, KT * ns * N)
            )
        wh = np.ascontiguousarray(wq[:, slots]).reshape(128, U * KT * OUT_DIM)
        in_maps.append({"xin": xh, "win": wh})
        core_meta.append((ids, slots))

    res = run_bass_kernel_spmd(
        nc, in_maps, list(range(NCORES)), **_cache.get("runkw", {})
    )
    _cache["last_result"] = res

    # --- gather, scale, bias, unpermute ---
    out = np.empty((B, N, OUT_DIM), dtype=np.float32)
    for c in range(NCORES):
        ids, slots = core_meta[c]
        oc = np.asarray(res.results[c]["o"], dtype=np.float32)  # [128, S*N]
        oc = oc.reshape(OUT_DIM, S, N).transpose(1, 2, 0)  # [S, N, O]
        # per-sample weight scale
        gcls = np.repeat(
            [g[0] for g in cores[c]], [g[2] for g in cores[c]]
        )  # class per sample
        scale = swu[np.array([cls_slot[c_] for c_ in gcls])][:, None] * sx[ids]
        oc = oc * scale[:, :, None]
        out[ids] = oc
    out += np.asarray(bias, dtype=np.float32)[yi][:, None, :]
    return out


# revision 18
# speedup vs baseline: 1.0546x; 1.0546x over previous
# MoE-routing kernel for Trainium2: out[b] = x[b] @ weight[y[b]] + bias[y[b]]
# x: [1024, 64, 1152] f32, y: [1024] int, weight: [1000, 1152, 128] f32,
# bias: [1000, 128] f32 -> out: [1024, 64, 128] f32.
#
# Strategy: sort samples by class so samples sharing a class share one weight
# load (1024 -> ~664 weight loads), then shard class-groups across 8 cores
# with an identical group-size multiset per core (SPMD requires one program:
# group sizes are capped at 4 and split until each size count divides by 8).
# Group sizes are interleaved so every DMA chunk moves a similar
# weight-bytes/sample mix, and chunk sizes ramp up (2,4,8,16,24...) and taper
# down (...,12,6,2) to shorten pipeline fill and drain.
# Per group (class c, m<=4 samples) the device runs a weight-stationary
# matmul: for each of 9 k-tiles, lhsT = W_c k-tile [128k, 128o] (stationary),
# rhs = x k-tile [128k, 64m] (moving), accumulating [128o, 64m] in PSUM f32.
# Both x and weights travel as fp8-e3m4 (per-row / per-class scales, applied
# with the bias on host); output is bf16. Host pre-packs partition-major,
# per-chunk-contiguous layouts so every DMA is one contiguous run per
# partition. x rides the ACT HWDGE ring, weights the SP HWDGE ring, outputs
# SWDGE (HBM-write receipts would bubble the input rings); the final tiny
# output chunk goes HWDGE to skip the Q7 drain in the kernel tail.
#
# Memory-bound: 23.7 MB/core of HBM traffic at the ~358 GB/s per-core
# roofline = 66 us floor + ~12.5 us fixed NEFF startup/shutdown; measured
# 78-85 us (median ~80 us) vs 183 us for the bf16 data-parallel baseline.
# Rel err 1.882e-2 (gate 2e-2), bit-deterministic across runs; fp8 e3m4
# quantization of both operands costs sqrt(2)*1.33e-2, verified to match a
# numpy simulation of the PE exactly.

import numpy as np

B, N, HIDDEN = 1024, 64, 1152
NUM_CLASSES = 1000
OUT_DIM = 128
KT = HIDDEN // 128  # 9 k-tiles
NCORES = 8
S = B // NCORES  # 128 samples per core
MAXM = 4         # max samples per group (PSUM free dim 64*m <= 256)

X_FP8 = True     # x dtype: False -> bf16, True -> fp8 e3m4 (per-row scale)
FP8_MAX = 15.5 * 0.96

_cache = {}


def _plan(y):
    """Group samples by class, canonicalize group sizes so every core gets an
    identical multiset, deal groups to cores. Returns (order, cores, sizes)
    where order sorts samples by class, cores[c] is a list of
    (cls, start, size) groups in canonical order (sizes interleaved
    uniformly), and sizes is the per-core size sequence (identical for all
    cores)."""
    order = np.argsort(y, kind="stable")
    ys = y[order]
    bysize = {s: [] for s in (4, 3, 2, 1)}
    i = 0
    while i < B:
        j = i
        while j < B and ys[j] == ys[i]:
            j += 1
        c, n, k = int(ys[i]), j - i, i
        while n > 0:
            t = min(n, MAXM)
            if n - t == 1 and t > 1:
                t -= 1  # avoid trailing singleton: prefer (3,2) over (4,1)
            bysize[t].append((c, k, t))
            k += t
            n -= t
        i = j
    # make each size count divisible by NCORES by splitting groups
    for s in (4, 3, 2):
        for _ in range(len(bysize[s]) % NCORES):
            c, st, _sz = bysize[s].pop()
            if s == 4:
                bysize[2] += [(c, st, 2), (c, st + 2, 2)]
            elif s == 3:
                bysize[2].append((c, st, 2))
                bysize[1].append((c, st + 2, 1))
            else:
                bysize[1] += [(c, st, 1), (c, st + 1, 1)]
    assert len(bysize[1]) % NCORES == 0
    percore = {s: len(bysize[s]) // NCORES for s in (4, 3, 2, 1)}
    # interleave sizes uniformly so every DMA chunk sees a similar
    # weight-bytes/sample mix (avoids a weight-DMA wall late in the run)
    slots = []
    for s, ns in percore.items():
        slots += [((j + 0.5) / ns, s) for j in range(ns)]
    slots.sort(key=lambda t: (t[0], t[1]))
    sizeseq = [s for _pos, s in slots]
    cores = [[] for _ in range(NCORES)]
    idx = {s: 0 for s in (4, 3, 2, 1)}
    for s in sizeseq:
        for c in range(NCORES):
            cores[c].append(bysize[s][idx[s] * NCORES + c])
        idx[s] += 1
    sizes = [g[2] for g in cores[0]]
    assert sum(sizes) == S
    return order, cores, sizes


def _chunks(sizes):
    """Split the canonical group sequence into DMA chunks. Returns a list of
    (g0, ng, s0, ns): group range and sample range per chunk."""
    targets = [2, 4, 8, 16] + [24] * 100
    out = []
    g0 = s0 = 0
    ti = 0
    while g0 < len(sizes):
        tgt = targets[min(ti, len(targets) - 1)]
        g, ns = g0, 0
        while g < len(sizes) and ns < tgt:
            ns += sizes[g]
            g += 1
        out.append((g0, g - g0, s0, ns))
        g0, s0, ti = g, s0 + ns, ti + 1
    # taper the final chunks so the pipeline drain is short
    for lim in (12, 6, 2):
        if out and out[-1][3] > lim + 2:
            g0, ng, s0, ns = out.pop()
            gs, acc = g0, 0
            while acc < ns - lim and gs < g0 + ng - 1:
                acc += sizes[gs]
                gs += 1
            out.append((g0, gs - g0, s0, acc))
            out.append((gs, g0 + ng - gs, s0 + acc, ns - acc))
    return out


def _build_nc(sizes, chunks, x_fp8):
    import concourse.bass as bass
    import concourse.mybir as mybir
    from concourse.tile import TileContext

    nc = bass.Bass()
    f32 = mybir.dt.float32
    bf16 = mybir.dt.bfloat16
    fp8 = mybir.dt.float8e3
    xdt = fp8 if x_fp8 else bf16
    U = len(sizes)
    # x is laid out per-chunk ([chunk][k][s][n]) so every x DMA is one
    # contiguous run per partition
    Xd = nc.declare_dram_parameter("xin", [128, KT * S * N], xdt, isOutput=False)
    Wd = nc.declare_dram_parameter("win", [128, U * KT * OUT_DIM], fp8, isOutput=False)
    Od = nc.declare_dram_parameter("o", [128, S * N], bf16, isOutput=True)

    with TileContext(nc) as tc:
        with (
            tc.tile_pool(name="xp", bufs=5) as xp,
            tc.tile_pool(name="wp", bufs=5) as wp,
            tc.tile_pool(name="op", bufs=4) as op,
            tc.tile_pool(name="pp", bufs=8, space="PSUM") as pp,
        ):
            for ci, (g0, ng, s0, ns) in enumerate(chunks):
                xt = xp.tile([128, KT * ns * N], xdt, tag="xt")
                nc.scalar.dma_start(
                    out=xt, in_=Xd[:, s0 * KT * N : (s0 + ns) * KT * N]
                )
                wt = wp.tile([128, ng * KT * OUT_DIM], fp8, tag="wt")
                nc.sync.dma_start(
                    out=wt,
                    in_=Wd[:, g0 * KT * OUT_DIM : (g0 + ng) * KT * OUT_DIM],
                )
                ot = op.tile([128, ns * N], bf16, tag="ot")
                spos = 0
                for gl in range(ng):
                    m = sizes[g0 + gl]
                    ps = pp.tile([128, MAXM * N], f32)
                    for k in range(KT):
                        nc.tensor.matmul(
                            ps[:, : m * N],
                            wt[:, (gl * KT + k) * OUT_DIM : (gl * KT + k + 1) * OUT_DIM],
                            xt[:, (k * ns + spos) * N : (k * ns + spos + m) * N],
                            start=(k == 0),
                            stop=(k == KT - 1),
                        )
                    nc.vector.tensor_copy(
                        ot[:, spos * N : (spos + m) * N], ps[:, : m * N]
                    )
                    spos += m
                # HBM-write receipts would bubble the input HWDGE rings, so
                # outputs ride SWDGE; the last (tiny) one goes HWDGE to
                # avoid the Q7 drain latency in the kernel tail.
                if ci == len(chunks) - 1:
                    nc.scalar.dma_start(out=Od[:, s0 * N : (s0 + ns) * N], in_=ot)
                else:
                    nc.gpsimd.dma_start(out=Od[:, s0 * N : (s0 + ns) * N], in_=ot)

    _split_excess_waits(nc)
    nc.finalize()
    _split_excess_waits(nc)
    return nc


def _split_excess_waits(nc, max_waits=1):
    # walrus codegen rejects instructions with >max sync waits; Tile's tail
    # drain can carry several. Hoist the excess onto preceding no-ops.
    import concourse.mybir as mybir

    for f in nc.m.functions:
        for b in f.blocks:
            i = 0
            while i < len(b.instructions):
                inst = b.instructions[i]
                si = inst.sync_info
                if si is not None and len(si.on_wait) > max_waits:
                    excess = list(si.on_wait[:-max_waits])
                    si.on_wait = list(si.on_wait[-max_waits:])
                    for w in excess:
                        nop = mybir.InstNoOp(
                            name=nc.get_next_instruction_name(),
                            engine=inst.engine,
                            sync_info=mybir.SyncInfo(on_wait=[w], on_update=[]),
                            bass_nofuse=True,
                        )
                        nc.register_instruction(nop)
                        b.instructions.insert(i, nop)
                        i += 1
                i += 1


def kernel(x, y, weight, bias):
    import ml_dtypes
    from concourse.bass_utils import run_bass_kernel_spmd

    e3 = ml_dtypes.float8_e3m4
    bf16 = ml_dtypes.bfloat16
    x = np.ascontiguousarray(x, dtype=np.float32)
    weight = np.ascontiguousarray(weight, dtype=np.float32)
    yi = np.asarray(y).astype(np.int64)

    order, cores, sizes = _plan(yi)
    chunks = _chunks(sizes)
    key = (tuple(sizes), X_FP8)
    if _cache.get("key") != key:
        _cache["nc"] = _build_nc(sizes, chunks, X_FP8)
        _cache["key"] = key
    nc = _cache["nc"]
    U = len(sizes)

    # --- quantize x ---
    if X_FP8:
        sx = np.abs(x).max(axis=2) / FP8_MAX  # [B, N]
        np.maximum(sx, 1e-30, out=sx)
        xq = (x / sx[:, :, None]).astype(e3)
    else:
        sx = np.ones((B, N), dtype=np.float32)
        xq = x.astype(bf16)

    # --- quantize weights for used classes (per-class scale) ---
    used = sorted({g[0] for core in cores for g in core})
    cls_slot = {c: i for i, c in enumerate(used)}
    wu = weight[used]  # [Ucls, H, O]
    swu = np.abs(wu).max(axis=(1, 2)) / FP8_MAX  # [Ucls]
    np.maximum(swu, 1e-30, out=swu)
    wq = (wu / swu[:, None, None]).astype(e3)
    # [Ucls, KT, 128, O] -> [128, Ucls, KT, O]
    wq = np.ascontiguousarray(
        wq.reshape(len(used), KT, 128, OUT_DIM).transpose(2, 0, 1, 3)
    )

    # --- per-core input layouts ---
    in_maps = []
    core_meta = []
    for core in cores:
        ids = np.concatenate(
            [order[st : st + m] for (_c, st, m) in core]
        )  # original sample ids, canonical order
        slots = np.array([cls_slot[c] for (c, _st, _m) in core])
        # x: per chunk [ns, N, KT, 128] -> [128, KT, ns, N], chunks concatenated
        xh = np.empty((128, KT * S * N), dtype=xq.dtype)
        for (_g0, _ng, s0, ns) in chunks:
            xh[:, s0 * KT * N : (s0 + ns) * KT * N] = (
                xq[ids[s0 : s0 + ns]]
                .reshape(ns, N, KT, 128)
                .transpose(3, 2, 0, 1)
                .reshape(128, KT * ns * N)
            )
        wh = np.ascontiguousarray(wq[:, slots]).reshape(128, U * KT * OUT_DIM)
        in_maps.append({"xin": xh, "win": wh})
        core_meta.append((ids, slots))

    res = run_bass_kernel_spmd(
        nc, in_maps, list(range(NCORES)), **_cache.get("runkw", {})
    )
    _cache["last_result"] = res

    # --- gather, scale, bias, unpermute ---
    out = np.empty((B, N, OUT_DIM), dtype=np.float32)
    for c in range(NCORES):
        ids, slots = core_meta[c]
        oc = np.asarray(res.results[c]["o"], dtype=np.float32)  # [128, S*N]
        oc = oc.reshape(OUT_DIM, S, N).transpose(1, 2, 0)  # [S, N, O]
        # per-sample weight scale
        gcls = np.repeat(
            [g[0] for g in cores[c]], [g[2] for g in cores[c]]
        )  # class per sample
        scale = swu[np.array([cls_slot[c_] for c_ in gcls])][:, None] * sx[ids]
        oc = oc * scale[:, :, None]
        out[ids] = oc
    out += np.asarray(bias, dtype=np.float32)[yi][:, None, :]
    return out
